# revision 2
# baseline (speedup 1.0000x reference)
"""Multi-head attention (b=2, s=2048, d_model=1024, H=16) on 8 TRN2 NeuronCores.

Head-sharded tensor parallelism: core c owns heads 2c and 2c+1 (a 128-wide
slice of the QKV feature dim). Each core computes its heads' Q/K/V, both
scores layouts (natural for softmax + attn output, transposed for the A@V
matmul), its partial x@W_O contribution, and writes its attn slice.
Host sums the 8 partial outputs and concatenates attn head slices.

TensorE-facing tensors are float32r (1 sign + 8 exp + 11 mantissa bits,
full-rate matmuls); softmax normalization happens on the natural-layout
attn tiles and, for the output path, is folded into per-head W_O partial
sums scaled per-partition by 1/rowsum.
"""

import numpy as np

import concourse.bass as bass
import concourse.tile as tile
from concourse import mybir
from concourse.bass import ts
from concourse.bass_utils import run_bass_kernel_spmd
from concourse.masks import make_identity

F32 = mybir.dt.float32
F32R = mybir.dt.float32r
AF = mybir.ActivationFunctionType

B = 2
S = 2048
D = 1024
H = 16
DK = 64
N_CORES = 8
T = B * S  # 4096 flattened tokens
DC = D // N_CORES  # 128 features per core (2 heads)

MAX_WAITS = 1  # this walrus accepts at most 1 sync-wait on CTRL-class ops


def _split_multi_waits(nc, max_waits=MAX_WAITS):
    """Move excess sem-waits onto preceding NoOps (walrus CTRL wait limit)."""
    ctr = [0]
    for f in nc.m.functions:
        for bb in f.blocks:
            new_insts = []
            for ins in bb.instructions:
                si = getattr(ins, "sync_info", None)
                if si is not None and len(si.on_wait) > max_waits:
                    waits = list(si.on_wait)
                    extra, keep = waits[:-max_waits], waits[-max_waits:]
                    for i in range(0, len(extra), max_waits):
                        ctr[0] += 1
                        nop = mybir.InstNoOp(
                            name=f"waitfix-{ctr[0]}", ins=[], outs=[]
                        )
                        nop.engine = ins.engine
                        nop.sync_info = mybir.SyncInfo(
                            on_wait=extra[i : i + max_waits], on_update=[]
                        )
                        nc.register_instruction(nop, overwrite=True)
                        new_insts.append(nop)
                    ins.sync_info = mybir.SyncInfo(
                        on_wait=keep, on_update=list(si.on_update)
                    )
                new_insts.append(ins)
            bb.instructions[:] = new_insts


def build_nc():
    nc = bass.Bass(trn_type="TRN2")

    xT = nc.dram_tensor("xT", [D, T], F32R, kind="ExternalInput")
    wqT = nc.dram_tensor("wqT", [D, DC], F32R, kind="ExternalInput")
    wkT = nc.dram_tensor("wkT", [D, DC], F32R, kind="ExternalInput")
    wvT = nc.dram_tensor("wvT", [D, DC], F32R, kind="ExternalInput")
    woT = nc.dram_tensor("woT", [DC, D], F32R, kind="ExternalInput")
    attn = nc.dram_tensor("attn", [B, 2, S, S], F32, kind="ExternalOutput")
    outp = nc.dram_tensor("outp", [T, D], F32, kind="ExternalOutput")

    with tile.TileContext(nc) as tc:
        with (
            tc.tile_pool(name="const", bufs=1) as const,
            tc.tile_pool(name="xt", bufs=2) as xtp,
            tc.tile_pool(name="p", bufs=3) as pp,
            tc.tile_pool(name="est", bufs=3) as estp,
            tc.tile_pool(name="vt", bufs=2) as vtp,
            tc.tile_pool(name="osb", bufs=3) as osbp,
            tc.tile_pool(name="small", bufs=4) as smallp,
            tc.tile_pool(name="pss", bufs=2, space="PSUM") as ps_s,
            tc.tile_pool(name="psst", bufs=2, space="PSUM") as ps_st,
            tc.tile_pool(name="psav", bufs=2, space="PSUM") as ps_av,
        ):
            # ---- constants / persistent buffers ----
            identity = const.tile([128, 128], F32, tag="ident")
            make_identity(nc, identity)

            wq_sb = const.tile([128, 8, DC], F32R, tag="wq")
            wk_sb = const.tile([128, 8, DC], F32R, tag="wk")
            wv_sb = const.tile([128, 8, DC], F32R, tag="wv")
            for wsb, wdram in ((wq_sb, wqT), (wk_sb, wkT), (wv_sb, wvT)):
                nc.sync.dma_start(
                    out=wsb, in_=wdram.rearrange("(c p) m -> p c m", p=128)
                )
            wo_sb = const.tile([128, D], F32R, tag="wo")
            nc.sync.dma_start(out=wo_sb, in_=woT[:, :])

            QT = const.tile([128, T], F32R, tag="qt")  # scaled by 1/8
            KT = const.tile([128, T], F32R, tag="kt")
            V = const.tile([128, T], F32R, tag="v")  # block g: V[g*128+p, dc]
            houT = const.tile([128, T], F32R, tag="hout")  # [dc, t] unnormalized
            rbuf = const.tile([128, 64], F32, tag="rbuf")  # 1/rowsum, col bh*16+qt

            # ---- phase A: projections QT, KT, V (+ on-chip transposes of V) ----
            xT_v = xT.rearrange("(c p) t -> p c t", p=128)
            for tb in range(8):
                xt = xtp.tile([128, 8, 512], F32R, tag="xt")
                nc.sync.dma_start(out=xt, in_=xT_v[:, :, ts(tb, 512)])
                for wsb, dest, scale in (
                    (wq_sb, QT, 0.125),
                    (wk_sb, KT, 1.0),
                ):
                    ps = ps_st.tile([128, 512], F32, tag="st")
                    for ch in range(8):
                        nc.tensor.matmul(
                            ps,
                            wsb[:, ch, :],
                            xt[:, ch, :],
                            start=(ch == 0),
                            stop=(ch == 7),
                        )
                    nc.scalar.activation(
                        dest[:, ts(tb, 512)], ps, AF.Copy, scale=scale
                    )
                # V^T block then PE-transpose into natural layout
                ps = ps_st.tile([128, 512], F32, tag="st")
                for ch in range(8):
                    nc.tensor.matmul(
                        ps,
                        wv_sb[:, ch, :],
                        xt[:, ch, :],
                        start=(ch == 0),
                        stop=(ch == 7),
                    )
                vt = vtp.tile([128, 512], F32, tag="vt")
                nc.scalar.copy(vt, ps)
                for j in range(4):
                    pt = ps_av.tile([128, 128], F32, tag="av")
                    nc.tensor.transpose(pt, vt[:, ts(j, 128)], identity)
                    g = tb * 4 + j
                    nc.scalar.copy(V[:, ts(g, 128)], pt)

            # ---- phase B: attention per (batch, local head) ----
            for bh in range(4):
                b, h = bh >> 1, bh & 1
                hoff = h * 64
                t0 = b * S

                # B1: natural scores, softmax, attn output, 1/rowsum
                for qt in range(16):
                    qs = QT[hoff : hoff + 64, t0 + qt * 128 : t0 + (qt + 1) * 128]
                    halves = []
                    for half in range(2):
                        psh = ps_s.tile([128, 1024], F32, tag="s")
                        for kc in range(2):
                            k0 = t0 + (half * 2 + kc) * 512
                            nc.tensor.matmul(
                                psh[:, ts(kc, 512)],
                                qs,
                                KT[hoff : hoff + 64, k0 : k0 + 512],
                                start=True,
                                stop=True,
                            )
                        halves.append(psh)
                    expP = pp.tile([128, S], F32, tag="p")
                    rsA = smallp.tile([128, 1], F32, tag="rs")
                    rsB = smallp.tile([128, 1], F32, tag="rs")
                    for half, rsx in ((0, rsA), (1, rsB)):
                        nc.scalar.activation(
                            expP[:, half * 1024 : (half + 1) * 1024],
                            halves[half],
                            AF.Exp,
                            accum_out=rsx,
                        )
                    rs = smallp.tile([128, 1], F32, tag="rsum")
                    nc.vector.tensor_add(rs, rsA, rsB)
                    rcol = rbuf[:, bh * 16 + qt : bh * 16 + qt + 1]
                    nc.vector.reciprocal(rcol, rs)
                    nc.vector.tensor_scalar_mul(expP, expP, rcol)
                    nc.sync.dma_start(
                        out=attn[b, h, qt * 128 : (qt + 1) * 128, :], in_=expP
                    )

                # B2: transposed scores + unnormalized A@V (software-pipelined)
                for qc in range(4):
                    pav = ps_av.tile([64, 512], F32, tag="av")
                    qslice = QT[hoff : hoff + 64, t0 + qc * 512 : t0 + (qc + 1) * 512]
                    ests = []
                    for kc in range(16):
                        pst = ps_st.tile([128, 512], F32, tag="st")
                        nc.tensor.matmul(
                            pst,
                            KT[hoff : hoff + 64, t0 + kc * 128 : t0 + (kc + 1) * 128],
                            qslice,
                            start=True,
                            stop=True,
                        )
                        est = estp.tile([128, 512], F32R, tag="est")
                        nc.scalar.activation(est, pst, AF.Exp)
                        ests.append(est)
                        if kc > 0:
                            g = b * 16 + (kc - 1)
                            nc.tensor.matmul(
                                pav,
                                V[:, g * 128 + hoff : g * 128 + hoff + 64],
                                ests[kc - 1],
                                start=(kc - 1 == 0),
                                stop=False,
                            )
                    g = b * 16 + 15
                    nc.tensor.matmul(
                        pav,
                        V[:, g * 128 + hoff : g * 128 + hoff + 64],
                        ests[15],
                        start=False,
                        stop=True,
                    )
                    nc.scalar.copy(
                        houT[hoff : hoff + 64, t0 + qc * 512 : t0 + (qc + 1) * 512],
                        pav,
                    )

                if h == 1:
                    # both heads of batch b done: per-head W_O partials,
                    # combined with per-partition 1/rowsum scaling.
                    for tt in range(16):
                        g = b * 16 + tt
                        r0 = rbuf[:, (b * 2) * 16 + tt : (b * 2) * 16 + tt + 1]
                        r1 = rbuf[:, (b * 2 + 1) * 16 + tt : (b * 2 + 1) * 16 + tt + 1]
                        for dch in range(2):
                            ph0 = ps_s.tile([128, 512], F32, tag="s")
                            nc.tensor.matmul(
                                ph0,
                                houT[0:64, ts(g, 128)],
                                wo_sb[0:64, ts(dch, 512)],
                                start=True,
                                stop=True,
                            )
                            ph1 = ps_s.tile([128, 512], F32, tag="s")
                            nc.tensor.matmul(
                                ph1,
                                houT[64:128, ts(g, 128)],
                                wo_sb[64:128, ts(dch, 512)],
                                start=True,
                                stop=True,
                            )
                            tmp = osbp.tile([128, 512], F32, tag="tmp")
                            nc.scalar.activation(tmp, ph0, AF.Copy, scale=r0)
                            osb = osbp.tile([128, 512], F32, tag="osb")
                            nc.vector.scalar_tensor_tensor(
                                osb,
                                ph1,
                                r1,
                                tmp,
                                op0=mybir.AluOpType.mult,
                                op1=mybir.AluOpType.add,
                            )
                            nc.sync.dma_start(
                                out=outp[
                                    g * 128 : (g + 1) * 128,
                                    dch * 512 : (dch + 1) * 512,
                                ],
                                in_=osb,
                            )

    _split_multi_waits(nc)
    return nc


_NC = None


def _get_nc():
    global _NC
    if _NC is None:
        _NC = build_nc()
    return _NC


def kernel(x, W_Q, W_K, W_V, W_O, _trace=False):
    x = np.asarray(x, dtype=np.float32)
    W_Q = np.asarray(W_Q, dtype=np.float32)
    W_K = np.asarray(W_K, dtype=np.float32)
    W_V = np.asarray(W_V, dtype=np.float32)
    W_O = np.asarray(W_O, dtype=np.float32)

    xT = np.ascontiguousarray(x.reshape(T, D).T)
    in_maps = []
    for c in range(N_CORES):
        sl = slice(c * DC, (c + 1) * DC)
        in_maps.append(
            {
                "xT": xT,
                "wqT": np.ascontiguousarray(W_Q[sl, :].T),
                "wkT": np.ascontiguousarray(W_K[sl, :].T),
                "wvT": np.ascontiguousarray(W_V[sl, :].T),
                "woT": np.ascontiguousarray(W_O[:, sl].T),
            }
        )

    nc = _get_nc()
    res = run_bass_kernel_spmd(
        nc, in_maps, core_ids=list(range(N_CORES)), trace=_trace
    )

    out = np.zeros((T, D), np.float32)
    attn = np.empty((B, H, S, S), np.float32)
    for c in range(N_CORES):
        out += res.results[c]["outp"]
        attn[:, 2 * c : 2 * c + 2] = res.results[c]["attn"]
    out = out.reshape(B, S, D)
    if _trace:
        return (out, attn), res
    return (out, attn)
